# revision 27
# baseline (speedup 1.0000x reference)
"""Trainium2 Bass kernel v8 for the GNN message-passing net.

Math (arange assoc => two fused MLP streams):
    out[0:n_con]      = head(con_mlp(con_node_features))
    out[n_con:n_var]  = head(var_mlp(var_node_features[n_con:n_var]))

Design (from HW micro-benchmarks; see trace analysis):
  - The wall: 4 PSUM->SBUF bias+ReLU drains per row, and only Act+DVE
    can read PSUM (Pool/GPSIMD and DMA cannot).  At 1 elem/cycle/
    partition this binds the kernel at ~2.8us per 1024-row group.
  - K<128 matmuls stream at 2 cycles/col, K=128 at 1; so L1 uses a
    K=128 zero-padded stationary: rows 0:2 = cW1, 2:4 = vW1, rows 4:5
    hold the (con,var) L1 biases against constant-1 feature rows --
    one LDW for both node types, bias-free r1 drain.
  - fp8 perf modes give no PE speedup on this toolchain (measured);
    everything stays f16.  LDWEIGHTS overlaps matmul streams.
  - PSUM: fresh [128,1024] tile per layer from a 3-buf pool (6 banks)
    + double-buffered p5 (2 banks).  Same-tile layer reuse (PTILES=1)
    slows drains ~20% (measured) -- avoid.
  - Drains: whole [128,1024] per layer: r1,r3 -> Act, r2,r4 -> DVE;
    sigmoid (quadrant-packed L5 via concurrent tile_position matmuls)
    -> Act, flushed at turn start where its input is already ready.
  - Emission: stage-descending turn order (L4-stage first so DVE's r4
    is ready earliest; L5-stage last), 7 groups in flight.
"""

import math
import os

import numpy as np

DIM = 128
TILE = 512
SUB = int(os.environ.get("K_SUB", "2"))          # subtiles per group
GT = SUB * TILE
N_CORES = 8
MMBUFS = int(os.environ.get("K_MMBUFS", "3"))    # psum group-tiles
P5BUFS = int(os.environ.get("K_P5BUFS", "2"))
P5W = int(os.environ.get("K_P5W", "512"))        # p5 tile width (512|1024)
INFLIGHT = int(os.environ.get("K_INFLIGHT", "7"))
PTILES = int(os.environ.get("K_PTILES", "4"))    # 1: tile/group, 4: tile/layer
DSPLIT = int(os.environ.get("K_DSPLIT", "1"))    # drain pieces per tile
NFEAT = int(os.environ.get("K_NFEAT", "6"))      # feature tile buffers
NACT = int(os.environ.get("K_NACT", "10"))       # h tile buffers
WARMUP = int(os.environ.get("K_WARMUP", "6"))
OLDFIRST = os.environ.get("K_OLDFIRST", "1") == "1"
R3ALT = int(os.environ.get("K_R3ALT", "16"))     # every Nth group r3->DVE

_NC_CACHE = {}


def _build_nc(ncon_sub, nvar_sub, zbias):
    """ncon_sub/nvar_sub: 512-row subtiles per core per type."""
    import concourse.mybir as mybir
    import concourse.tile as tile
    from concourse import bacc

    dt = mybir.dt
    f32 = dt.float32
    f16 = dt.float16
    AF = mybir.ActivationFunctionType
    ALU = mybir.AluOpType

    nsub = ncon_sub + nvar_sub
    ngroups = nsub // SUB
    assert nsub % SUB == 0
    ncols = nsub * TILE

    nc = bacc.Bacc()

    feat = nc.dram_tensor("feat", [6, ncols], f16, kind="ExternalInput")
    blob16 = nc.dram_tensor("blob16", [DIM, 800], f16, kind="ExternalInput")
    blob32 = nc.dram_tensor("blob32", [DIM, 7], f32, kind="ExternalInput")
    out = nc.dram_tensor("out", [ncols], f32, kind="ExternalOutput")

    with tile.TileContext(nc) as tc:
        with (
            tc.tile_pool(name="const", bufs=1) as cpool,
            tc.tile_pool(name="feat", bufs=NFEAT) as fpool,
            tc.tile_pool(name="acts", bufs=NACT) as apool,
            tc.tile_pool(name="sig", bufs=2) as spool,
            tc.tile_pool(name="mm", bufs=MMBUFS, space="PSUM") as mmpool,
            tc.tile_pool(name="p5", bufs=P5BUFS, space="PSUM") as p5pool,
        ):
            b16 = cpool.tile([DIM, 800], f16, tag="b16")
            nc.sync.dma_start(b16[:, :], blob16[:, :])
            b32 = cpool.tile([DIM, 7], f32, tag="b32")
            nc.sync.dma_start(b32[:, :], blob32[:, :])

            w1_t = b16[:, 0:128]       # rows 0:2 W1c, rows 2:4 W1v, rest 0
            wmc_t = b16[:, 256:384]
            wmv_t = b16[:, 384:512]
            ww2_t = b16[:, 512:640]
            ww3_t = b16[:, 640:768]
            ww4_t = b16[:, 768:800]
            bb1c_t = b32[:, 0:1]
            bb1v_t = b32[:, 1:2]
            bbmc_t = b32[:, 2:3]
            bbmv_t = b32[:, 3:4]
            bb2_t = b32[:, 4:5]
            bb3_t = b32[:, 5:6]
            bb4_t = b32[:, 6:7]

            ftiles = []
            for i in range(NFEAT):
                ft = fpool.tile([DIM, GT], f16, tag="feat", name=f"ft{i}")
                eng = (nc.gpsimd, nc.vector)[i % 2]
                eng.memset(ft[:, :], 0.0)
                ftiles.append(ft)

            # subtile type: 0=con (first ncon_sub), 1=var
            def sub_type(si):
                return 0 if si < ncon_sub else 1

            COHORT = 4 * (P5W // TILE)  # subtiles per p5 tile
            sig_state = {"n": 0, "g0": None, "p5": None}

            def flush_sig():
                nq = sig_state["n"]
                if nq == 0:
                    return
                g0 = sig_state["g0"]
                p5 = sig_state["p5"]
                nparts = 32 * min(nq, 4)
                nhalf = (nq + 3) // 4  # column halves used
                sg = spool.tile([DIM, P5W], f32, tag="sig")
                nc.scalar.activation(
                    sg[:nparts, : nhalf * TILE], p5[:nparts, : nhalf * TILE],
                    AF.Sigmoid, bias=bb4_t[:nparts, :],
                )
                for h in range(nhalf):
                    cnt = min(nq - 4 * h, 4)
                    pc = 32 * (cnt - 1) + 1
                    nc.gpsimd.dma_start(
                        out[g0 + 4 * h * TILE:
                            g0 + (4 * h + cnt) * TILE].rearrange(
                                "(a b) -> a b", b=TILE),
                        sg[0:pc:32, h * TILE:(h + 1) * TILE],
                    )
                sig_state["n"] = 0
                sig_state["g0"] = None
                sig_state["p5"] = None

            def _bias(b_key, ty):
                if b_key == "b1":
                    return bb1c_t if ty == 0 else bb1v_t
                if b_key == "bm":
                    return bbmc_t if ty == 0 else bbmv_t
                return bb2_t if b_key == "b2" else bb3_t

            def _drain(eng, t, p, g, b_key):
                """bias+relu drain p -> t, split only at a con/var
                boundary (per-type biases)."""
                s0 = g * SUB
                segs = []
                start = 0
                while start < SUB:
                    ty = sub_type(s0 + start)
                    run = 1
                    while start + run < SUB and sub_type(s0 + start + run) == ty:
                        run += 1
                    segs.append((start * TILE, run * TILE, ty))
                    start += run
                if DSPLIT > 1:
                    nsegs = []
                    for so, sw, ty in segs:
                        nsub = sw // TILE
                        if nsub >= DSPLIT:
                            step = (nsub // DSPLIT) * TILE
                            c = so
                            while c < so + sw:
                                w = min(step, so + sw - c)
                                nsegs.append((c, w, ty))
                                c += w
                        else:
                            nsegs.append((so, sw, ty))
                    segs = nsegs
                for so, sw, ty in segs:
                    bias = _bias(b_key, ty)
                    if eng == "act":
                        nc.scalar.activation(
                            t[:, so:so + sw], p[:, so:so + sw],
                            AF.Relu, bias=bias,
                        )
                    elif zbias:
                        nc.vector.tensor_scalar(
                            t[:, so:so + sw], p[:, so:so + sw],
                            0.0, None, ALU.max,
                        )
                    else:
                        nc.vector.tensor_scalar(
                            t[:, so:so + sw], p[:, so:so + sw],
                            bias, 0.0, ALU.add, ALU.max,
                        )

            def emit_stage(st):
                m, li = st["m"], st["li"]
                g = m["g"]
                if li == 0:
                    # L1: K=128 zero-padded stationary serves both types
                    p = mmpool.tile([DIM, GT], f32, tag="mm")
                    st["p"] = p
                    ft = st["ft"]
                    for s in range(SUB):
                        nc.tensor.matmul(
                            p[:, s * TILE:(s + 1) * TILE], w1_t,
                            ft[:, s * TILE:(s + 1) * TILE],
                            start=True, stop=True,
                        )
                    t = apool.tile([DIM, GT], f16, tag="acts")
                    nc.scalar.activation(t[:, :], p[:, :], AF.Relu)
                    st["t"] = t
                elif li < 4:
                    w_sel = (
                        (lambda ty: wmc_t if ty == 0 else wmv_t),
                        (lambda ty: ww2_t),
                        (lambda ty: ww3_t),
                    )[li - 1]
                    b_key = ("bm", "b2", "b3")[li - 1]
                    if PTILES == 4 or (PTILES == 2 and li == 2):
                        p = mmpool.tile([DIM, GT], f32, tag="mm")
                        st["p"] = p
                    else:
                        p = st["p"]
                    for s in range(SUB):
                        nc.tensor.matmul(
                            p[:, s * TILE:(s + 1) * TILE],
                            w_sel(sub_type(g * SUB + s)),
                            st["t"][:, s * TILE:(s + 1) * TILE],
                            start=True, stop=True,
                        )
                    t = apool.tile([DIM, GT], f16, tag="acts")
                    eng = "act" if li == 2 else "dve"
                    if li == 2 and R3ALT and g % R3ALT == R3ALT - 1:
                        eng = "dve"
                    _drain(eng, t, p, g, b_key)
                    st["t"] = t
                else:
                    # L5: quadrant-packed into p5
                    for s in range(SUB):
                        si = g * SUB + s
                        ci = si % COHORT
                        q = 32 * (ci % 4)
                        h = ci // 4
                        if ci == 0:
                            if sig_state["n"]:
                                flush_sig()
                            sig_state["g0"] = si * TILE
                            p5t = p5pool.tile([DIM, P5W], f32, tag="p5")
                            sig_state["p5"] = p5t
                        nc.tensor.matmul(
                            sig_state["p5"][q:q + 32,
                                            h * TILE:(h + 1) * TILE],
                            ww4_t,
                            st["t"][:, s * TILE:(s + 1) * TILE],
                            start=True, stop=True,
                            tile_position=(0, q),
                        )
                        sig_state["n"] += 1
                st["li"] = li + 1

            if WARMUP:
                wp = mmpool.tile([DIM, GT], f32, tag="mm")
                for _ in range(WARMUP):
                    nc.tensor.matmul(
                        wp[:, 0:TILE], b16[:, 0:128], b16[:, 288:800],
                        start=True, stop=True,
                    )

            groups = [{"g": g, "g0": g * GT} for g in range(ngroups)]

            stream = iter(groups)
            fidx = [0]

            def new_st():
                m = next(stream, None)
                if m is None:
                    return None
                ft = ftiles[fidx[0] % NFEAT]
                fidx[0] += 1
                nc.sync.dma_start(
                    ft[0:6, :], feat[:, m["g0"]:m["g0"] + GT]
                )
                return {"m": m, "li": 0, "ft": ft}

            active = []
            pending = True
            while active or pending:
                if pending and len(active) < INFLIGHT:
                    st = new_st()
                    if st is None:
                        pending = False
                    else:
                        active.append(st)
                done = []
                if sig_state["n"] >= COHORT:
                    flush_sig()
                if OLDFIRST:
                    # L4-stage first (feeds DVE earliest), L5-stage last
                    _prio = {4: 9, 3: 0, 2: 1, 1: 2, 0: 3}
                    order = sorted(active, key=lambda s: _prio[s["li"]])
                else:
                    order = list(reversed(active))
                for st in order:
                    emit_stage(st)
                    if st["li"] == 5:
                        done.append(st)
                for st in done:
                    active.remove(st)
            flush_sig()

    nc.compile()
    return nc


def _make_in_maps(inputs, ncon_sub, nvar_sub, *_ignored):
    f32 = np.float32
    f16 = np.float16
    cf = np.asarray(inputs["con_node_features"], f32)
    vf = np.asarray(inputs["var_node_features"], f32)
    n_con = cf.shape[0]
    n_var = vf.shape[0]

    W1 = np.asarray(inputs["W1"], f32)
    b1 = np.asarray(inputs["b1"], f32)
    mc = np.asarray(inputs["cW2"], f32) @ W1
    bmc = np.asarray(inputs["cb2"], f32) @ W1 + b1
    mv = np.asarray(inputs["vW2"], f32) @ W1
    bmv = np.asarray(inputs["vb2"], f32) @ W1 + b1

    ncp = n_con // N_CORES            # 50000
    nvp = (n_var - n_con) // N_CORES  # 25000
    ncon_cols = ncon_sub * TILE
    nvar_cols = nvar_sub * TILE
    ncols = ncon_cols + nvar_cols

    cfT = cf.T  # [2, n_con]
    vfT = vf[n_con:].T

    blob16 = np.zeros((DIM, 800), f16)
    blob16[0:2, 0:128] = np.asarray(inputs["cW1"], f32).astype(f16)
    blob16[2:4, 0:128] = np.asarray(inputs["vW1"], f32).astype(f16)
    blob16[4, 0:128] = np.asarray(inputs["cb1"], f32).astype(f16)
    blob16[5, 0:128] = np.asarray(inputs["vb1"], f32).astype(f16)
    blob16[:, 256:384] = mc.astype(f16)
    blob16[:, 384:512] = mv.astype(f16)
    blob16[:, 512:640] = np.asarray(inputs["W2"], f32).astype(f16)
    blob16[:, 640:768] = np.asarray(inputs["W3"], f32).astype(f16)
    blob16[:, 768:800] = np.repeat(
        np.asarray(inputs["W4"], f32).reshape(DIM, 1), 32, axis=1
    ).astype(f16)
    blob32 = np.zeros((DIM, 7), f32)
    blob32[:, 0] = np.asarray(inputs["cb1"], f32)
    blob32[:, 1] = np.asarray(inputs["vb1"], f32)
    blob32[:, 2] = bmc
    blob32[:, 3] = bmv
    blob32[:, 4] = np.asarray(inputs["b2"], f32)
    blob32[:, 5] = np.asarray(inputs["b3"], f32)
    blob32[:, 6] = np.asarray(inputs["b4"], f32).reshape(-1)[0]

    shared = {"blob16": blob16, "blob32": blob32}
    in_maps = []
    for i in range(N_CORES):
        feat = np.zeros((6, ncols), f16)
        c = cfT[:, i * ncp:(i + 1) * ncp]
        feat[0:2, : c.shape[1]] = c
        feat[4, :ncon_cols] = 1.0
        v = vfT[:, i * nvp:(i + 1) * nvp]
        feat[2:4, ncon_cols:ncon_cols + v.shape[1]] = v
        feat[5, ncon_cols:] = 1.0
        m = dict(shared)
        m["feat"] = feat
        in_maps.append(m)
    return in_maps


def _reference_numpy(inputs):
    f32 = np.float32

    def mlp2(x, W1, b1, W2, b2):
        return np.maximum(x @ W1 + b1, 0.0) @ W2 + b2

    vf = np.asarray(inputs["var_node_features"], f32)
    cf = np.asarray(inputs["con_node_features"], f32)
    av = np.asarray(inputs["assoc_var"])
    ac = np.asarray(inputs["assoc_con"])
    n = mlp2(vf, inputs["vW1"], inputs["vb1"], inputs["vW2"], inputs["vb2"])
    e = mlp2(cf, inputs["cW1"], inputs["cb1"], inputs["cW2"], inputs["cb2"])
    x = np.zeros((np.asarray(inputs["node_types"]).shape[0], n.shape[-1]), f32)
    x[av] = n
    x[ac] = e
    x = x[av]
    x = np.maximum(x @ inputs["W1"] + inputs["b1"], 0.0)
    x = np.maximum(x @ inputs["W2"] + inputs["b2"], 0.0)
    x = np.maximum(x @ inputs["W3"] + inputs["b3"], 0.0)
    x = x @ inputs["W4"] + inputs["b4"]
    return (1.0 / (1.0 + np.exp(-x))).astype(f32).squeeze(-1)


def kernel(**inputs):
    from concourse.bass_utils import run_bass_kernel_spmd

    cf = np.asarray(inputs["con_node_features"])
    vf = np.asarray(inputs["var_node_features"])
    av = np.asarray(inputs["assoc_var"])
    ac = np.asarray(inputs["assoc_con"])
    n_con = cf.shape[0]
    n_var = vf.shape[0]

    fast = (
        n_con <= n_var
        and n_con % N_CORES == 0
        and (n_var - n_con) % N_CORES == 0
        and av.shape[0] == n_var
        and ac.shape[0] == n_con
        and np.array_equal(av, np.arange(n_var, dtype=av.dtype))
        and np.array_equal(ac, np.arange(n_con, dtype=ac.dtype))
    )
    if not fast:
        return _reference_numpy(inputs)

    ncp = n_con // N_CORES
    nvp = (n_var - n_con) // N_CORES
    ncon_sub = math.ceil(ncp / TILE)
    nvar_sub = math.ceil(nvp / TILE)
    # pad subtile total to a multiple of SUB
    while (ncon_sub + nvar_sub) % SUB:
        nvar_sub += 1

    in_maps = _make_in_maps(inputs, ncon_sub, nvar_sub)
    b32 = in_maps[0]["blob32"]
    # DVE-drained layers (r2: bm, r4: b3) can skip the bias operand
    zbias = bool(np.all(b32[:, 2:4] == 0.0) and np.all(b32[:, 5] == 0.0))

    key = (ncon_sub, nvar_sub, zbias)
    if key not in _NC_CACHE:
        _NC_CACHE[key] = _build_nc(ncon_sub, nvar_sub, zbias)
    nc = _NC_CACHE[key]
    res = run_bass_kernel_spmd(nc, in_maps, core_ids=list(range(N_CORES)))

    ncon_cols = ncon_sub * TILE
    out = np.empty(n_var, np.float32)
    oc = np.concatenate([r["out"][:ncp] for r in res.results])
    ov = np.concatenate(
        [r["out"][ncon_cols:ncon_cols + nvp] for r in res.results]
    )
    out[:n_con] = oc
    out[n_con:] = ov
    return out


# revision 28
# speedup vs baseline: 1.0005x; 1.0005x over previous
"""Trainium2 Bass kernel v8 for the GNN message-passing net.

Math (arange assoc => two fused MLP streams):
    out[0:n_con]      = head(con_mlp(con_node_features))
    out[n_con:n_var]  = head(var_mlp(var_node_features[n_con:n_var]))

Design (from HW micro-benchmarks; see trace analysis):
  - The wall: 4 PSUM->SBUF bias+ReLU drains per row, and only Act+DVE
    can read PSUM (Pool/GPSIMD and DMA cannot).  At 1 elem/cycle/
    partition this binds the kernel at ~2.8us per 1024-row group.
  - K<128 matmuls stream at 2 cycles/col, K=128 at 1; so L1 uses a
    K=128 zero-padded stationary: rows 0:2 = cW1, 2:4 = vW1, rows 4:5
    hold the (con,var) L1 biases against constant-1 feature rows --
    one LDW for both node types, bias-free r1 drain.
  - fp8 perf modes give no PE speedup on this toolchain (measured);
    everything stays f16.  LDWEIGHTS overlaps matmul streams.
  - PSUM: fresh [128,1024] tile per layer from a 3-buf pool (6 banks)
    + double-buffered p5 (2 banks).  Same-tile layer reuse (PTILES=1)
    slows drains ~20% (measured) -- avoid.
  - Drains: whole [128,1024] per layer: r1,r3 -> Act, r2,r4 -> DVE;
    sigmoid (quadrant-packed L5 via concurrent tile_position matmuls)
    -> Act, flushed at turn start where its input is already ready.
  - Emission: stage-descending turn order (L4-stage first so DVE's r4
    is ready earliest; L5-stage last), 7 groups in flight.
"""

import math
import os

import numpy as np

DIM = 128
TILE = 512
SUB = int(os.environ.get("K_SUB", "2"))          # subtiles per group
GT = SUB * TILE
N_CORES = 8
MMBUFS = int(os.environ.get("K_MMBUFS", "3"))    # psum group-tiles
P5BUFS = int(os.environ.get("K_P5BUFS", "2"))
P5W = int(os.environ.get("K_P5W", "512"))        # p5 tile width (512|1024)
INFLIGHT = int(os.environ.get("K_INFLIGHT", "7"))
PTILES = int(os.environ.get("K_PTILES", "4"))    # 1: tile/group, 4: tile/layer
DSPLIT = int(os.environ.get("K_DSPLIT", "1"))    # drain pieces per tile
NFEAT = int(os.environ.get("K_NFEAT", "6"))      # feature tile buffers
NACT = int(os.environ.get("K_NACT", "10"))       # h tile buffers
WARMUP = int(os.environ.get("K_WARMUP", "6"))
OLDFIRST = os.environ.get("K_OLDFIRST", "1") == "1"
R3ALT = int(os.environ.get("K_R3ALT", "0"))     # every Nth group r3->DVE

_NC_CACHE = {}


def _build_nc(ncon_sub, nvar_sub, zbias):
    """ncon_sub/nvar_sub: 512-row subtiles per core per type."""
    import concourse.mybir as mybir
    import concourse.tile as tile
    from concourse import bacc

    dt = mybir.dt
    f32 = dt.float32
    f16 = dt.float16
    AF = mybir.ActivationFunctionType
    ALU = mybir.AluOpType

    nsub = ncon_sub + nvar_sub
    ngroups = nsub // SUB
    assert nsub % SUB == 0
    ncols = nsub * TILE

    nc = bacc.Bacc()

    feat = nc.dram_tensor("feat", [6, ncols], f16, kind="ExternalInput")
    blob16 = nc.dram_tensor("blob16", [DIM, 800], f16, kind="ExternalInput")
    blob32 = nc.dram_tensor("blob32", [DIM, 7], f32, kind="ExternalInput")
    out = nc.dram_tensor("out", [ncols], f32, kind="ExternalOutput")

    with tile.TileContext(nc) as tc:
        with (
            tc.tile_pool(name="const", bufs=1) as cpool,
            tc.tile_pool(name="feat", bufs=NFEAT) as fpool,
            tc.tile_pool(name="acts", bufs=NACT) as apool,
            tc.tile_pool(name="sig", bufs=2) as spool,
            tc.tile_pool(name="mm", bufs=MMBUFS, space="PSUM") as mmpool,
            tc.tile_pool(name="p5", bufs=P5BUFS, space="PSUM") as p5pool,
        ):
            b16 = cpool.tile([DIM, 800], f16, tag="b16")
            nc.sync.dma_start(b16[:, :], blob16[:, :])
            b32 = cpool.tile([DIM, 7], f32, tag="b32")
            nc.sync.dma_start(b32[:, :], blob32[:, :])

            w1_t = b16[:, 0:128]       # rows 0:2 W1c, rows 2:4 W1v, rest 0
            wmc_t = b16[:, 256:384]
            wmv_t = b16[:, 384:512]
            ww2_t = b16[:, 512:640]
            ww3_t = b16[:, 640:768]
            ww4_t = b16[:, 768:800]
            bb1c_t = b32[:, 0:1]
            bb1v_t = b32[:, 1:2]
            bbmc_t = b32[:, 2:3]
            bbmv_t = b32[:, 3:4]
            bb2_t = b32[:, 4:5]
            bb3_t = b32[:, 5:6]
            bb4_t = b32[:, 6:7]

            ftiles = []
            for i in range(NFEAT):
                ft = fpool.tile([DIM, GT], f16, tag="feat", name=f"ft{i}")
                eng = (nc.gpsimd, nc.vector)[i % 2]
                eng.memset(ft[:, :], 0.0)
                ftiles.append(ft)

            # subtile type: 0=con (first ncon_sub), 1=var
            def sub_type(si):
                return 0 if si < ncon_sub else 1

            COHORT = 4 * (P5W // TILE)  # subtiles per p5 tile
            sig_state = {"n": 0, "g0": None, "p5": None}

            def flush_sig():
                nq = sig_state["n"]
                if nq == 0:
                    return
                g0 = sig_state["g0"]
                p5 = sig_state["p5"]
                nparts = 32 * min(nq, 4)
                nhalf = (nq + 3) // 4  # column halves used
                sg = spool.tile([DIM, P5W], f32, tag="sig")
                nc.scalar.activation(
                    sg[:nparts, : nhalf * TILE], p5[:nparts, : nhalf * TILE],
                    AF.Sigmoid, bias=bb4_t[:nparts, :],
                )
                for h in range(nhalf):
                    cnt = min(nq - 4 * h, 4)
                    pc = 32 * (cnt - 1) + 1
                    nc.gpsimd.dma_start(
                        out[g0 + 4 * h * TILE:
                            g0 + (4 * h + cnt) * TILE].rearrange(
                                "(a b) -> a b", b=TILE),
                        sg[0:pc:32, h * TILE:(h + 1) * TILE],
                    )
                sig_state["n"] = 0
                sig_state["g0"] = None
                sig_state["p5"] = None

            def _bias(b_key, ty):
                if b_key == "b1":
                    return bb1c_t if ty == 0 else bb1v_t
                if b_key == "bm":
                    return bbmc_t if ty == 0 else bbmv_t
                return bb2_t if b_key == "b2" else bb3_t

            def _drain(eng, t, p, g, b_key):
                """bias+relu drain p -> t, split only at a con/var
                boundary (per-type biases)."""
                s0 = g * SUB
                segs = []
                start = 0
                while start < SUB:
                    ty = sub_type(s0 + start)
                    run = 1
                    while start + run < SUB and sub_type(s0 + start + run) == ty:
                        run += 1
                    segs.append((start * TILE, run * TILE, ty))
                    start += run
                if DSPLIT > 1:
                    nsegs = []
                    for so, sw, ty in segs:
                        nsub = sw // TILE
                        if nsub >= DSPLIT:
                            step = (nsub // DSPLIT) * TILE
                            c = so
                            while c < so + sw:
                                w = min(step, so + sw - c)
                                nsegs.append((c, w, ty))
                                c += w
                        else:
                            nsegs.append((so, sw, ty))
                    segs = nsegs
                for so, sw, ty in segs:
                    bias = _bias(b_key, ty)
                    if eng == "act":
                        nc.scalar.activation(
                            t[:, so:so + sw], p[:, so:so + sw],
                            AF.Relu, bias=bias,
                        )
                    elif zbias:
                        nc.vector.tensor_scalar(
                            t[:, so:so + sw], p[:, so:so + sw],
                            0.0, None, ALU.max,
                        )
                    else:
                        nc.vector.tensor_scalar(
                            t[:, so:so + sw], p[:, so:so + sw],
                            bias, 0.0, ALU.add, ALU.max,
                        )

            def emit_stage(st):
                m, li = st["m"], st["li"]
                g = m["g"]
                if li == 0:
                    # L1: K=128 zero-padded stationary serves both types
                    p = mmpool.tile([DIM, GT], f32, tag="mm")
                    st["p"] = p
                    ft = st["ft"]
                    for s in range(SUB):
                        nc.tensor.matmul(
                            p[:, s * TILE:(s + 1) * TILE], w1_t,
                            ft[:, s * TILE:(s + 1) * TILE],
                            start=True, stop=True,
                        )
                    t = apool.tile([DIM, GT], f16, tag="acts")
                    nc.scalar.activation(t[:, :], p[:, :], AF.Relu)
                    st["t"] = t
                elif li < 4:
                    w_sel = (
                        (lambda ty: wmc_t if ty == 0 else wmv_t),
                        (lambda ty: ww2_t),
                        (lambda ty: ww3_t),
                    )[li - 1]
                    b_key = ("bm", "b2", "b3")[li - 1]
                    if PTILES == 4 or (PTILES == 2 and li == 2):
                        p = mmpool.tile([DIM, GT], f32, tag="mm")
                        st["p"] = p
                    else:
                        p = st["p"]
                    for s in range(SUB):
                        nc.tensor.matmul(
                            p[:, s * TILE:(s + 1) * TILE],
                            w_sel(sub_type(g * SUB + s)),
                            st["t"][:, s * TILE:(s + 1) * TILE],
                            start=True, stop=True,
                        )
                    t = apool.tile([DIM, GT], f16, tag="acts")
                    eng = "act" if li == 2 else "dve"
                    if li == 2 and R3ALT and g % R3ALT == R3ALT - 1:
                        eng = "dve"
                    _drain(eng, t, p, g, b_key)
                    st["t"] = t
                else:
                    # L5: quadrant-packed into p5
                    for s in range(SUB):
                        si = g * SUB + s
                        ci = si % COHORT
                        q = 32 * (ci % 4)
                        h = ci // 4
                        if ci == 0:
                            if sig_state["n"]:
                                flush_sig()
                            sig_state["g0"] = si * TILE
                            p5t = p5pool.tile([DIM, P5W], f32, tag="p5")
                            sig_state["p5"] = p5t
                        nc.tensor.matmul(
                            sig_state["p5"][q:q + 32,
                                            h * TILE:(h + 1) * TILE],
                            ww4_t,
                            st["t"][:, s * TILE:(s + 1) * TILE],
                            start=True, stop=True,
                            tile_position=(0, q),
                        )
                        sig_state["n"] += 1
                st["li"] = li + 1

            if WARMUP:
                wp = mmpool.tile([DIM, GT], f32, tag="mm")
                for _ in range(WARMUP):
                    nc.tensor.matmul(
                        wp[:, 0:TILE], b16[:, 0:128], b16[:, 288:800],
                        start=True, stop=True,
                    )

            groups = [{"g": g, "g0": g * GT} for g in range(ngroups)]

            stream = iter(groups)
            fidx = [0]

            def new_st():
                m = next(stream, None)
                if m is None:
                    return None
                ft = ftiles[fidx[0] % NFEAT]
                fidx[0] += 1
                nc.sync.dma_start(
                    ft[0:6, :], feat[:, m["g0"]:m["g0"] + GT]
                )
                return {"m": m, "li": 0, "ft": ft}

            active = []
            pending = True
            while active or pending:
                if pending and len(active) < INFLIGHT:
                    st = new_st()
                    if st is None:
                        pending = False
                    else:
                        active.append(st)
                done = []
                if sig_state["n"] >= COHORT:
                    flush_sig()
                if OLDFIRST:
                    # L4-stage first (feeds DVE earliest), L5-stage last
                    _prio = {4: 9, 3: 0, 2: 1, 1: 2, 0: 3}
                    order = sorted(active, key=lambda s: _prio[s["li"]])
                else:
                    order = list(reversed(active))
                for st in order:
                    emit_stage(st)
                    if st["li"] == 5:
                        done.append(st)
                for st in done:
                    active.remove(st)
            flush_sig()

    nc.compile()
    return nc


def _make_in_maps(inputs, ncon_sub, nvar_sub, *_ignored):
    f32 = np.float32
    f16 = np.float16
    cf = np.asarray(inputs["con_node_features"], f32)
    vf = np.asarray(inputs["var_node_features"], f32)
    n_con = cf.shape[0]
    n_var = vf.shape[0]

    W1 = np.asarray(inputs["W1"], f32)
    b1 = np.asarray(inputs["b1"], f32)
    mc = np.asarray(inputs["cW2"], f32) @ W1
    bmc = np.asarray(inputs["cb2"], f32) @ W1 + b1
    mv = np.asarray(inputs["vW2"], f32) @ W1
    bmv = np.asarray(inputs["vb2"], f32) @ W1 + b1

    ncp = n_con // N_CORES            # 50000
    nvp = (n_var - n_con) // N_CORES  # 25000
    ncon_cols = ncon_sub * TILE
    nvar_cols = nvar_sub * TILE
    ncols = ncon_cols + nvar_cols

    cfT = cf.T  # [2, n_con]
    vfT = vf[n_con:].T

    blob16 = np.zeros((DIM, 800), f16)
    blob16[0:2, 0:128] = np.asarray(inputs["cW1"], f32).astype(f16)
    blob16[2:4, 0:128] = np.asarray(inputs["vW1"], f32).astype(f16)
    blob16[4, 0:128] = np.asarray(inputs["cb1"], f32).astype(f16)
    blob16[5, 0:128] = np.asarray(inputs["vb1"], f32).astype(f16)
    blob16[:, 256:384] = mc.astype(f16)
    blob16[:, 384:512] = mv.astype(f16)
    blob16[:, 512:640] = np.asarray(inputs["W2"], f32).astype(f16)
    blob16[:, 640:768] = np.asarray(inputs["W3"], f32).astype(f16)
    blob16[:, 768:800] = np.repeat(
        np.asarray(inputs["W4"], f32).reshape(DIM, 1), 32, axis=1
    ).astype(f16)
    blob32 = np.zeros((DIM, 7), f32)
    blob32[:, 0] = np.asarray(inputs["cb1"], f32)
    blob32[:, 1] = np.asarray(inputs["vb1"], f32)
    blob32[:, 2] = bmc
    blob32[:, 3] = bmv
    blob32[:, 4] = np.asarray(inputs["b2"], f32)
    blob32[:, 5] = np.asarray(inputs["b3"], f32)
    blob32[:, 6] = np.asarray(inputs["b4"], f32).reshape(-1)[0]

    shared = {"blob16": blob16, "blob32": blob32}
    in_maps = []
    for i in range(N_CORES):
        feat = np.zeros((6, ncols), f16)
        c = cfT[:, i * ncp:(i + 1) * ncp]
        feat[0:2, : c.shape[1]] = c
        feat[4, :ncon_cols] = 1.0
        v = vfT[:, i * nvp:(i + 1) * nvp]
        feat[2:4, ncon_cols:ncon_cols + v.shape[1]] = v
        feat[5, ncon_cols:] = 1.0
        m = dict(shared)
        m["feat"] = feat
        in_maps.append(m)
    return in_maps


def _reference_numpy(inputs):
    f32 = np.float32

    def mlp2(x, W1, b1, W2, b2):
        return np.maximum(x @ W1 + b1, 0.0) @ W2 + b2

    vf = np.asarray(inputs["var_node_features"], f32)
    cf = np.asarray(inputs["con_node_features"], f32)
    av = np.asarray(inputs["assoc_var"])
    ac = np.asarray(inputs["assoc_con"])
    n = mlp2(vf, inputs["vW1"], inputs["vb1"], inputs["vW2"], inputs["vb2"])
    e = mlp2(cf, inputs["cW1"], inputs["cb1"], inputs["cW2"], inputs["cb2"])
    x = np.zeros((np.asarray(inputs["node_types"]).shape[0], n.shape[-1]), f32)
    x[av] = n
    x[ac] = e
    x = x[av]
    x = np.maximum(x @ inputs["W1"] + inputs["b1"], 0.0)
    x = np.maximum(x @ inputs["W2"] + inputs["b2"], 0.0)
    x = np.maximum(x @ inputs["W3"] + inputs["b3"], 0.0)
    x = x @ inputs["W4"] + inputs["b4"]
    return (1.0 / (1.0 + np.exp(-x))).astype(f32).squeeze(-1)


def kernel(**inputs):
    from concourse.bass_utils import run_bass_kernel_spmd

    cf = np.asarray(inputs["con_node_features"])
    vf = np.asarray(inputs["var_node_features"])
    av = np.asarray(inputs["assoc_var"])
    ac = np.asarray(inputs["assoc_con"])
    n_con = cf.shape[0]
    n_var = vf.shape[0]

    fast = (
        n_con <= n_var
        and n_con % N_CORES == 0
        and (n_var - n_con) % N_CORES == 0
        and av.shape[0] == n_var
        and ac.shape[0] == n_con
        and np.array_equal(av, np.arange(n_var, dtype=av.dtype))
        and np.array_equal(ac, np.arange(n_con, dtype=ac.dtype))
    )
    if not fast:
        return _reference_numpy(inputs)

    ncp = n_con // N_CORES
    nvp = (n_var - n_con) // N_CORES
    ncon_sub = math.ceil(ncp / TILE)
    nvar_sub = math.ceil(nvp / TILE)
    # pad subtile total to a multiple of SUB
    while (ncon_sub + nvar_sub) % SUB:
        nvar_sub += 1

    in_maps = _make_in_maps(inputs, ncon_sub, nvar_sub)
    b32 = in_maps[0]["blob32"]
    # DVE-drained layers (r2: bm, r4: b3) can skip the bias operand
    zbias = bool(np.all(b32[:, 2:4] == 0.0) and np.all(b32[:, 5] == 0.0))

    key = (ncon_sub, nvar_sub, zbias)
    if key not in _NC_CACHE:
        _NC_CACHE[key] = _build_nc(ncon_sub, nvar_sub, zbias)
    nc = _NC_CACHE[key]
    res = run_bass_kernel_spmd(nc, in_maps, core_ids=list(range(N_CORES)))

    ncon_cols = ncon_sub * TILE
    out = np.empty(n_var, np.float32)
    oc = np.concatenate([r["out"][:ncp] for r in res.results])
    ov = np.concatenate(
        [r["out"][ncon_cols:ncon_cols + nvp] for r in res.results]
    )
    out[:n_con] = oc
    out[n_con:] = ov
    return out


# revision 29
# speedup vs baseline: 1.0123x; 1.0118x over previous
"""Trainium2 Bass kernel v8 for the GNN message-passing net.

Math (arange assoc => two fused MLP streams):
    out[0:n_con]      = head(con_mlp(con_node_features))
    out[n_con:n_var]  = head(var_mlp(var_node_features[n_con:n_var]))

Design (from HW micro-benchmarks; see trace analysis):
  - The wall: 4 PSUM->SBUF bias+ReLU drains per row, and only Act+DVE
    can read PSUM (Pool/GPSIMD and DMA cannot).  At 1 elem/cycle/
    partition this binds the kernel at ~2.8us per 1024-row group.
  - K<128 matmuls stream at 2 cycles/col, K=128 at 1; so L1 uses a
    K=128 zero-padded stationary: rows 0:2 = cW1, 2:4 = vW1, rows 4:5
    hold the (con,var) L1 biases against constant-1 feature rows --
    one LDW for both node types, bias-free r1 drain.
  - fp8 perf modes give no PE speedup on this toolchain (measured);
    everything stays f16.  LDWEIGHTS overlaps matmul streams.
  - PSUM: fresh [128,1024] tile per layer from a 3-buf pool (6 banks)
    + double-buffered p5 (2 banks).  Same-tile layer reuse (PTILES=1)
    slows drains ~20% (measured) -- avoid.
  - Drains: whole [128,1024] per layer: r1,r3 -> Act, r2,r4 -> DVE;
    sigmoid (quadrant-packed L5 via concurrent tile_position matmuls)
    -> Act, flushed at turn start where its input is already ready.
  - Emission: stage-descending turn order (L4-stage first so DVE's r4
    is ready earliest; L5-stage last), 7 groups in flight.
"""

import math
import os

import numpy as np

DIM = 128
TILE = 512
SUB = int(os.environ.get("K_SUB", "2"))          # subtiles per group
GT = SUB * TILE
N_CORES = 8
MMBUFS = int(os.environ.get("K_MMBUFS", "3"))    # psum group-tiles
P5BUFS = int(os.environ.get("K_P5BUFS", "2"))
P5W = int(os.environ.get("K_P5W", "512"))        # p5 tile width (512|1024)
INFLIGHT = int(os.environ.get("K_INFLIGHT", "7"))
PTILES = int(os.environ.get("K_PTILES", "4"))    # 1: tile/group, 4: tile/layer
DSPLIT = int(os.environ.get("K_DSPLIT", "1"))    # drain pieces per tile
NFEAT = int(os.environ.get("K_NFEAT", "6"))      # feature tile buffers
NACT = int(os.environ.get("K_NACT", "10"))       # h tile buffers
WARMUP = int(os.environ.get("K_WARMUP", "6"))
OLDFIRST = os.environ.get("K_OLDFIRST", "1") == "1"
R3ALT = int(os.environ.get("K_R3ALT", "0"))     # every Nth group r3->DVE

_NC_CACHE = {}


def _build_nc(ncon_sub, nvar_sub, zbias):
    """ncon_sub/nvar_sub: 512-row subtiles per core per type."""
    import concourse.mybir as mybir
    import concourse.tile as tile
    from concourse import bacc

    dt = mybir.dt
    f32 = dt.float32
    f16 = dt.float16
    AF = mybir.ActivationFunctionType
    ALU = mybir.AluOpType

    nsub = ncon_sub + nvar_sub
    ngroups = nsub // SUB
    assert nsub % SUB == 0
    ncols = nsub * TILE

    nc = bacc.Bacc()

    feat = nc.dram_tensor("feat", [6, ncols], f16, kind="ExternalInput")
    blob16 = nc.dram_tensor("blob16", [DIM, 800], f16, kind="ExternalInput")
    blob32 = nc.dram_tensor("blob32", [DIM, 7], f32, kind="ExternalInput")
    out = nc.dram_tensor("out", [ncols], f32, kind="ExternalOutput")

    with tile.TileContext(nc) as tc:
        with (
            tc.tile_pool(name="const", bufs=1) as cpool,
            tc.tile_pool(name="feat", bufs=NFEAT) as fpool,
            tc.tile_pool(name="acts", bufs=NACT) as apool,
            tc.tile_pool(name="sig", bufs=2) as spool,
            tc.tile_pool(name="mm", bufs=MMBUFS, space="PSUM") as mmpool,
            tc.tile_pool(name="p5", bufs=P5BUFS, space="PSUM") as p5pool,
        ):
            b16 = cpool.tile([DIM, 800], f16, tag="b16")
            nc.sync.dma_start(b16[:, :], blob16[:, :])
            b32 = cpool.tile([DIM, 7], f32, tag="b32")
            nc.sync.dma_start(b32[:, :], blob32[:, :])

            w1_t = b16[:, 0:128]       # rows 0:2 W1c, rows 2:4 W1v, rest 0
            wmc_t = b16[:, 256:384]
            wmv_t = b16[:, 384:512]
            ww2_t = b16[:, 512:640]
            ww3_t = b16[:, 640:768]
            ww4_t = b16[:, 768:800]
            bb1c_t = b32[:, 0:1]
            bb1v_t = b32[:, 1:2]
            bbmc_t = b32[:, 2:3]
            bbmv_t = b32[:, 3:4]
            bb2_t = b32[:, 4:5]
            bb3_t = b32[:, 5:6]
            bb4_t = b32[:, 6:7]

            ftiles = []
            for i in range(NFEAT):
                ft = fpool.tile([DIM, GT], f16, tag="feat", name=f"ft{i}")
                eng = (nc.gpsimd, nc.vector)[i % 2]
                eng.memset(ft[:, :], 0.0)
                ftiles.append(ft)

            # subtile type: 0=con (first ncon_sub), 1=var
            def sub_type(si):
                return 0 if si < ncon_sub else 1

            COHORT = 4 * (P5W // TILE)  # subtiles per p5 tile
            sig_state = {"n": 0, "g0": None, "p5": None}

            def flush_sig():
                nq = sig_state["n"]
                if nq == 0:
                    return
                g0 = sig_state["g0"]
                p5 = sig_state["p5"]
                nparts = 32 * min(nq, 4)
                nhalf = (nq + 3) // 4  # column halves used
                sg = spool.tile([DIM, P5W], f32, tag="sig")
                nc.scalar.activation(
                    sg[:nparts, : nhalf * TILE], p5[:nparts, : nhalf * TILE],
                    AF.Sigmoid, bias=bb4_t[:nparts, :],
                )
                for h in range(nhalf):
                    cnt = min(nq - 4 * h, 4)
                    pc = 32 * (cnt - 1) + 1
                    nc.gpsimd.dma_start(
                        out[g0 + 4 * h * TILE:
                            g0 + (4 * h + cnt) * TILE].rearrange(
                                "(a b) -> a b", b=TILE),
                        sg[0:pc:32, h * TILE:(h + 1) * TILE],
                    )
                sig_state["n"] = 0
                sig_state["g0"] = None
                sig_state["p5"] = None

            def _bias(b_key, ty):
                if b_key == "b1":
                    return bb1c_t if ty == 0 else bb1v_t
                if b_key == "bm":
                    return bbmc_t if ty == 0 else bbmv_t
                return bb2_t if b_key == "b2" else bb3_t

            def _drain(eng, t, p, g, b_key):
                """bias+relu drain p -> t, split only at a con/var
                boundary (per-type biases)."""
                s0 = g * SUB
                segs = []
                start = 0
                while start < SUB:
                    ty = sub_type(s0 + start)
                    run = 1
                    while start + run < SUB and sub_type(s0 + start + run) == ty:
                        run += 1
                    segs.append((start * TILE, run * TILE, ty))
                    start += run
                if DSPLIT > 1:
                    nsegs = []
                    for so, sw, ty in segs:
                        nsub = sw // TILE
                        if nsub >= DSPLIT:
                            step = (nsub // DSPLIT) * TILE
                            c = so
                            while c < so + sw:
                                w = min(step, so + sw - c)
                                nsegs.append((c, w, ty))
                                c += w
                        else:
                            nsegs.append((so, sw, ty))
                    segs = nsegs
                for so, sw, ty in segs:
                    bias = _bias(b_key, ty)
                    if eng == "act":
                        nc.scalar.activation(
                            t[:, so:so + sw], p[:, so:so + sw],
                            AF.Relu, bias=bias,
                        )
                    elif zbias:
                        nc.vector.tensor_scalar(
                            t[:, so:so + sw], p[:, so:so + sw],
                            0.0, None, ALU.max,
                        )
                    else:
                        nc.vector.tensor_scalar(
                            t[:, so:so + sw], p[:, so:so + sw],
                            bias, 0.0, ALU.add, ALU.max,
                        )

            def emit_stage(st):
                m, li = st["m"], st["li"]
                g = m["g"]
                if li == 0:
                    # L1: K=128 zero-padded stationary serves both types
                    p = mmpool.tile([DIM, GT], f32, tag="mm")
                    st["p"] = p
                    ft = st["ft"]
                    for s in range(SUB):
                        nc.tensor.matmul(
                            p[:, s * TILE:(s + 1) * TILE], w1_t,
                            ft[:, s * TILE:(s + 1) * TILE],
                            start=True, stop=True,
                        )
                    t = apool.tile([DIM, GT], f16, tag="acts")
                    nc.scalar.activation(t[:, :], p[:, :], AF.Relu)
                    st["t"] = t
                elif li < 4:
                    w_sel = (
                        (lambda ty: wmc_t if ty == 0 else wmv_t),
                        (lambda ty: ww2_t),
                        (lambda ty: ww3_t),
                    )[li - 1]
                    b_key = ("bm", "b2", "b3")[li - 1]
                    if PTILES == 4 or (PTILES == 2 and li == 2):
                        p = mmpool.tile([DIM, GT], f32, tag="mm")
                        st["p"] = p
                    else:
                        p = st["p"]
                    for s in range(SUB):
                        nc.tensor.matmul(
                            p[:, s * TILE:(s + 1) * TILE],
                            w_sel(sub_type(g * SUB + s)),
                            st["t"][:, s * TILE:(s + 1) * TILE],
                            start=True, stop=True,
                        )
                    t = apool.tile([DIM, GT], f16, tag="acts")
                    eng = "act" if li == 2 else "dve"
                    if li == 2 and R3ALT and g % R3ALT == R3ALT - 1:
                        eng = "dve"
                    _drain(eng, t, p, g, b_key)
                    st["t"] = t
                else:
                    # L5: quadrant-packed into p5
                    for s in range(SUB):
                        si = g * SUB + s
                        ci = si % COHORT
                        q = 32 * (ci % 4)
                        h = ci // 4
                        if ci == 0:
                            if sig_state["n"]:
                                flush_sig()
                            sig_state["g0"] = si * TILE
                            p5t = p5pool.tile([DIM, P5W], f32, tag="p5")
                            sig_state["p5"] = p5t
                        nc.tensor.matmul(
                            sig_state["p5"][q:q + 32,
                                            h * TILE:(h + 1) * TILE],
                            ww4_t,
                            st["t"][:, s * TILE:(s + 1) * TILE],
                            start=True, stop=True,
                            tile_position=(0, q),
                        )
                        sig_state["n"] += 1
                st["li"] = li + 1

            if WARMUP:
                wp = mmpool.tile([DIM, GT], f32, tag="mm")
                for _ in range(WARMUP):
                    nc.tensor.matmul(
                        wp[:, 0:TILE], b16[:, 0:128], b16[:, 288:800],
                        start=True, stop=True,
                    )

            groups = [{"g": g, "g0": g * GT} for g in range(ngroups)]

            stream = iter(groups)
            dma_ptr = [0]

            def issue_feat_dma():
                gi = dma_ptr[0]
                if gi >= len(groups):
                    return
                dma_ptr[0] += 1
                m = groups[gi]
                ft = ftiles[gi % NFEAT]
                nc.sync.dma_start(
                    ft[0:6, :], feat[:, m["g0"]:m["g0"] + GT]
                )

            # prefetch two groups ahead of admission
            issue_feat_dma()
            issue_feat_dma()

            def new_st():
                m = next(stream, None)
                if m is None:
                    return None
                issue_feat_dma()
                return {"m": m, "li": 0, "ft": ftiles[m["g"] % NFEAT]}

            active = []
            pending = True
            while active or pending:
                if pending and len(active) < INFLIGHT:
                    st = new_st()
                    if st is None:
                        pending = False
                    else:
                        active.append(st)
                done = []
                if sig_state["n"] >= COHORT:
                    flush_sig()
                if OLDFIRST:
                    # L4-stage first (feeds DVE earliest), L5-stage last
                    _prio = {4: 9, 3: 0, 2: 1, 1: 2, 0: 3}
                    order = sorted(active, key=lambda s: _prio[s["li"]])
                else:
                    order = list(reversed(active))
                for st in order:
                    emit_stage(st)
                    if st["li"] == 5:
                        done.append(st)
                for st in done:
                    active.remove(st)
            flush_sig()

    nc.compile()
    return nc


def _make_in_maps(inputs, ncon_sub, nvar_sub, *_ignored):
    f32 = np.float32
    f16 = np.float16
    cf = np.asarray(inputs["con_node_features"], f32)
    vf = np.asarray(inputs["var_node_features"], f32)
    n_con = cf.shape[0]
    n_var = vf.shape[0]

    W1 = np.asarray(inputs["W1"], f32)
    b1 = np.asarray(inputs["b1"], f32)
    mc = np.asarray(inputs["cW2"], f32) @ W1
    bmc = np.asarray(inputs["cb2"], f32) @ W1 + b1
    mv = np.asarray(inputs["vW2"], f32) @ W1
    bmv = np.asarray(inputs["vb2"], f32) @ W1 + b1

    ncp = n_con // N_CORES            # 50000
    nvp = (n_var - n_con) // N_CORES  # 25000
    ncon_cols = ncon_sub * TILE
    nvar_cols = nvar_sub * TILE
    ncols = ncon_cols + nvar_cols

    cfT = cf.T  # [2, n_con]
    vfT = vf[n_con:].T

    blob16 = np.zeros((DIM, 800), f16)
    blob16[0:2, 0:128] = np.asarray(inputs["cW1"], f32).astype(f16)
    blob16[2:4, 0:128] = np.asarray(inputs["vW1"], f32).astype(f16)
    blob16[4, 0:128] = np.asarray(inputs["cb1"], f32).astype(f16)
    blob16[5, 0:128] = np.asarray(inputs["vb1"], f32).astype(f16)
    blob16[:, 256:384] = mc.astype(f16)
    blob16[:, 384:512] = mv.astype(f16)
    blob16[:, 512:640] = np.asarray(inputs["W2"], f32).astype(f16)
    blob16[:, 640:768] = np.asarray(inputs["W3"], f32).astype(f16)
    blob16[:, 768:800] = np.repeat(
        np.asarray(inputs["W4"], f32).reshape(DIM, 1), 32, axis=1
    ).astype(f16)
    blob32 = np.zeros((DIM, 7), f32)
    blob32[:, 0] = np.asarray(inputs["cb1"], f32)
    blob32[:, 1] = np.asarray(inputs["vb1"], f32)
    blob32[:, 2] = bmc
    blob32[:, 3] = bmv
    blob32[:, 4] = np.asarray(inputs["b2"], f32)
    blob32[:, 5] = np.asarray(inputs["b3"], f32)
    blob32[:, 6] = np.asarray(inputs["b4"], f32).reshape(-1)[0]

    shared = {"blob16": blob16, "blob32": blob32}
    in_maps = []
    for i in range(N_CORES):
        feat = np.zeros((6, ncols), f16)
        c = cfT[:, i * ncp:(i + 1) * ncp]
        feat[0:2, : c.shape[1]] = c
        feat[4, :ncon_cols] = 1.0
        v = vfT[:, i * nvp:(i + 1) * nvp]
        feat[2:4, ncon_cols:ncon_cols + v.shape[1]] = v
        feat[5, ncon_cols:] = 1.0
        m = dict(shared)
        m["feat"] = feat
        in_maps.append(m)
    return in_maps


def _reference_numpy(inputs):
    f32 = np.float32

    def mlp2(x, W1, b1, W2, b2):
        return np.maximum(x @ W1 + b1, 0.0) @ W2 + b2

    vf = np.asarray(inputs["var_node_features"], f32)
    cf = np.asarray(inputs["con_node_features"], f32)
    av = np.asarray(inputs["assoc_var"])
    ac = np.asarray(inputs["assoc_con"])
    n = mlp2(vf, inputs["vW1"], inputs["vb1"], inputs["vW2"], inputs["vb2"])
    e = mlp2(cf, inputs["cW1"], inputs["cb1"], inputs["cW2"], inputs["cb2"])
    x = np.zeros((np.asarray(inputs["node_types"]).shape[0], n.shape[-1]), f32)
    x[av] = n
    x[ac] = e
    x = x[av]
    x = np.maximum(x @ inputs["W1"] + inputs["b1"], 0.0)
    x = np.maximum(x @ inputs["W2"] + inputs["b2"], 0.0)
    x = np.maximum(x @ inputs["W3"] + inputs["b3"], 0.0)
    x = x @ inputs["W4"] + inputs["b4"]
    return (1.0 / (1.0 + np.exp(-x))).astype(f32).squeeze(-1)


def kernel(**inputs):
    from concourse.bass_utils import run_bass_kernel_spmd

    cf = np.asarray(inputs["con_node_features"])
    vf = np.asarray(inputs["var_node_features"])
    av = np.asarray(inputs["assoc_var"])
    ac = np.asarray(inputs["assoc_con"])
    n_con = cf.shape[0]
    n_var = vf.shape[0]

    fast = (
        n_con <= n_var
        and n_con % N_CORES == 0
        and (n_var - n_con) % N_CORES == 0
        and av.shape[0] == n_var
        and ac.shape[0] == n_con
        and np.array_equal(av, np.arange(n_var, dtype=av.dtype))
        and np.array_equal(ac, np.arange(n_con, dtype=ac.dtype))
    )
    if not fast:
        return _reference_numpy(inputs)

    ncp = n_con // N_CORES
    nvp = (n_var - n_con) // N_CORES
    ncon_sub = math.ceil(ncp / TILE)
    nvar_sub = math.ceil(nvp / TILE)
    # pad subtile total to a multiple of SUB
    while (ncon_sub + nvar_sub) % SUB:
        nvar_sub += 1

    in_maps = _make_in_maps(inputs, ncon_sub, nvar_sub)
    b32 = in_maps[0]["blob32"]
    # DVE-drained layers (r2: bm, r4: b3) can skip the bias operand
    zbias = bool(np.all(b32[:, 2:4] == 0.0) and np.all(b32[:, 5] == 0.0))

    key = (ncon_sub, nvar_sub, zbias)
    if key not in _NC_CACHE:
        _NC_CACHE[key] = _build_nc(ncon_sub, nvar_sub, zbias)
    nc = _NC_CACHE[key]
    res = run_bass_kernel_spmd(nc, in_maps, core_ids=list(range(N_CORES)))

    ncon_cols = ncon_sub * TILE
    out = np.empty(n_var, np.float32)
    oc = np.concatenate([r["out"][:ncp] for r in res.results])
    ov = np.concatenate(
        [r["out"][ncon_cols:ncon_cols + nvp] for r in res.results]
    )
    out[:n_con] = oc
    out[n_con:] = ov
    return out
